# revision 8
# baseline (speedup 1.0000x reference)
import os

import numpy as np
import ml_dtypes

# nn_GateModLinear: B=4096, M=8 experts, DI=DO=2048, LN eps=1e-5.
#   h[b,m,i] = sum_j Ws[m,i,j] x[b,j]
#   Wx = gW * sum_m pW[b,m] h[b,m,:]
#   z  = Wx + gb * (pb @ bs)
#   out = ELU(LayerNorm(z))
#
# Strategy: data-parallel over batch across 8 NeuronCores (512 rows each).
# Per core: one bf16 [512,16384]x[16384,2048] matmul on TensorE, streamed
# over W once; per-expert PSUM tiles are mixed with pW on ScalarE (per-
# partition scale) + VectorE adds; bias path is a K=128 zero-padded matmul;
# gating, LayerNorm and ELU run on VectorE/ScalarE. ELU uses
#   elu(y) = relu(y) + min(exp(y)-1, 0).
# The last output-column pass is split into two batch halves so half of the
# LN/ELU epilogue overlaps the tail of the matmul stream.

B, M, DI, DO = 4096, 8, 2048, 2048
NCORES = 8
BS = B // NCORES      # 512 batch rows per core
BT = BS // 128        # 4 b-tiles of 128 (partition dim)
IC = DO // 512        # 4 output chunks of 512
JT = DI // 128        # 16 contraction tiles of 128 per expert
LN_EPS = 1e-5

_PROGRAM = None          # cached bass.Bass program
LAST_EXEC_NS = None
LAST_RESULTS = None


def _build_program():
    import concourse.bacc as bacc
    import concourse.bass as bass
    import concourse.tile as tile
    import concourse.mybir as mybir

    f32 = mybir.dt.float32
    bf16 = mybir.dt.bfloat16
    AF = mybir.ActivationFunctionType
    OP = mybir.AluOpType

    nc = bacc.Bacc("TRN2")

    # [ic, m, jj, jt, ii]: W[(m, jt*128+jj), ic*512+ii] = Ws[m, i, j]
    w_d = nc.dram_tensor("w", [IC, M, 128, JT, 512], bf16, kind="ExternalInput")
    # [jj, jt, b]: xT for this core, k=jt*128+jj on partitions
    xt_d = nc.dram_tensor("xt", [128, JT, BS], bf16, kind="ExternalInput")
    pw_d = nc.dram_tensor("pw", [BS, M], f32, kind="ExternalInput")
    # [:, :DO] = bs zero-padded to 128 rows; [:, DO:] = pb.T zero-padded.
    pbs_d = nc.dram_tensor("pbs", [128, DO + BS], f32, kind="ExternalInput")
    gw_d = nc.dram_tensor("gw", [BS, DO], f32, kind="ExternalInput")
    gb_d = nc.dram_tensor("gb", [BS, DO], f32, kind="ExternalInput")
    out_d = nc.dram_tensor("out", [BS, DO], f32, kind="ExternalOutput")

    with tile.TileContext(nc) as tc:
        with (
            tc.tile_pool(name="const", bufs=1) as cpool,
            tc.tile_pool(name="wpool", bufs=2) as wpool,
            tc.tile_pool(name="acc", bufs=1) as apool,
            tc.tile_pool(name="mix", bufs=3) as mpool,
            tc.tile_pool(name="gio", bufs=4) as gpool,
            tc.tile_pool(name="epi", bufs=1) as epool,
            tc.tile_pool(name="psum", bufs=2, space=bass.MemorySpace.PSUM) as pspool,
        ):
            # ---- constants / persistent tiles ----
            # pbs first: the bias matmuls depend only on it, so PE starts
            # (and HAM-warms) as early as possible.
            pbs_s = cpool.tile([128, DO + BS], f32)
            nc.sync.dma_start(out=pbs_s[:], in_=pbs_d[:])
            xt_s = cpool.tile([128, JT, BS], bf16)
            nc.sync.dma_start(out=xt_s[:], in_=xt_d[:])
            pw_t = []
            for bt in range(BT):
                t = cpool.tile([128, M], f32, name=f"pw{bt}", tag=f"pw{bt}")
                nc.sync.dma_start(out=t[:], in_=pw_d[bt * 128:(bt + 1) * 128, :])
                pw_t.append(t)
            eps_t = cpool.tile([128, 1], f32, name="eps", tag="eps")
            nc.vector.memset(eps_t[:], LN_EPS)

            z_t = [apool.tile([128, DO], f32, name=f"z{bt}", tag=f"z{bt}")
                   for bt in range(BT)]
            wx_t = [apool.tile([128, DO], f32, name=f"wx{bt}", tag=f"wx{bt}")
                    for bt in range(BT)]
            st_t = [apool.tile([128, IC, 6], f32, name=f"st{bt}", tag=f"st{bt}")
                    for bt in range(BT)]

            # ---- phase A: z = gb * (pb @ bs) (bias path, K padded to 128) ----
            # gb loads ride the Scalar HWDGE ring so the Sync ring is free for
            # the W stream.
            for bt in range(BT):
                for ic in range(IC):
                    ps = pspool.tile([128, 512], f32, name=f"ps{bt}", tag=f"b{bt}")
                    nc.tensor.matmul(
                        ps[:],
                        pbs_s[:, DO + bt * 128:DO + (bt + 1) * 128],
                        pbs_s[:, ic * 512:(ic + 1) * 512],
                        start=True,
                        stop=True,
                    )
                    gbt = gpool.tile([128, 512], f32, name="gbt", tag="gb")
                    nc.scalar.dma_start(
                        out=gbt[:],
                        in_=gb_d[bt * 128:(bt + 1) * 128, ic * 512:(ic + 1) * 512],
                    )
                    nc.vector.tensor_mul(
                        z_t[bt][:, ic * 512:(ic + 1) * 512], gbt[:], ps[:]
                    )

            def mm_group(ic, m, bts):
                """One (ic, m) accumulation group over the given b-tiles."""
                w_s = wpool.tile([128, JT, 512], bf16, name="ws", tag="w")
                nc.sync.dma_start(out=w_s[:], in_=w_d[ic, m])
                pss = {
                    bt: pspool.tile([128, 512], f32, name=f"ps{bt}", tag=f"b{bt}")
                    for bt in bts
                }
                for jt in range(JT):
                    for bt in bts:
                        nc.tensor.matmul(
                            pss[bt][:],
                            xt_s[:, jt, bt * 128:(bt + 1) * 128],
                            w_s[:, jt, :],
                            start=(jt == 0),
                            stop=(jt == JT - 1),
                        )
                for bt in bts:
                    wxs = wx_t[bt][:, ic * 512:(ic + 1) * 512]
                    if m == 0:
                        nc.scalar.activation(
                            wxs, pss[bt][:], AF.Copy, scale=pw_t[bt][:, 0:1]
                        )
                    else:
                        tmp = mpool.tile([128, 512], f32, name="tmp", tag="tmp")
                        nc.scalar.activation(
                            tmp[:], pss[bt][:], AF.Copy, scale=pw_t[bt][:, m:m + 1]
                        )
                        nc.vector.tensor_add(wxs, wxs, tmp[:])

            def combine(ic, bts):
                """z[:, ic] += gW * wx and bn_stats for the chunk."""
                for bt in bts:
                    gwt = gpool.tile([128, 512], f32, name="gwt", tag="gw")
                    nc.scalar.dma_start(
                        out=gwt[:],
                        in_=gw_d[bt * 128:(bt + 1) * 128, ic * 512:(ic + 1) * 512],
                    )
                    wxs = wx_t[bt][:, ic * 512:(ic + 1) * 512]
                    zs = z_t[bt][:, ic * 512:(ic + 1) * 512]
                    tmp2 = mpool.tile([128, 512], f32, name="tmp2", tag="tmp2")
                    nc.vector.tensor_mul(tmp2[:], gwt[:], wxs)
                    nc.vector.tensor_add(zs, zs, tmp2[:])
                    nc.vector.bn_stats(st_t[bt][:, ic, :], zs)

            def epilogue(bts):
                """LayerNorm + ELU + store for the given b-tiles."""
                mvs, rstds = {}, {}
                for bt in bts:
                    mv = mpool.tile([128, 2], f32, name="mv", tag=f"mv{bt}")
                    nc.vector.bn_aggr(mv[:], st_t[bt][:])
                    mvs[bt] = mv
                for bt in bts:  # grouped: one ACT Sqrt table load
                    sd = mpool.tile([128, 1], f32, name="sd", tag=f"sd{bt}")
                    nc.scalar.activation(sd[:], mvs[bt][:, 1:2], AF.Sqrt,
                                         bias=eps_t[:])
                    rstds[bt] = sd
                for bt in bts:
                    nc.vector.reciprocal(rstds[bt][:], rstds[bt][:])
                ys, es = {}, {}
                for bt in bts:
                    y = epool.tile([128, DO], f32, name="y", tag=f"y{bt % 2}")
                    nc.vector.tensor_scalar(
                        out=y[:],
                        in0=z_t[bt][:],
                        scalar1=mvs[bt][:, 0:1],
                        scalar2=rstds[bt][:],
                        op0=OP.subtract,
                        op1=OP.mult,
                    )
                    ys[bt] = y
                for bt in bts:  # grouped: one ACT Exp table load
                    e = epool.tile([128, DO], f32, name="e", tag=f"e{bt % 2}")
                    nc.scalar.activation(e[:], ys[bt][:], AF.Exp)
                    es[bt] = e
                for bt in bts:
                    y, e = ys[bt], es[bt]
                    nc.vector.tensor_scalar_max(y[:], y[:], 0.0)
                    nc.vector.tensor_scalar(
                        out=e[:],
                        in0=e[:],
                        scalar1=1.0,
                        scalar2=1.0,
                        op0=OP.min,
                        op1=OP.subtract,
                    )
                    nc.vector.tensor_add(y[:], y[:], e[:])
                    nc.scalar.dma_start(
                        out=out_d[bt * 128:(bt + 1) * 128, :], in_=y[:]
                    )

            # ---- phase B: main matmul + expert mix + gate + stats ----
            for ic in range(IC - 1):
                for m in range(M):
                    mm_group(ic, m, range(BT))
                combine(ic, range(BT))
            # Last column chunk in two b-tile halves: the first half's
            # combine + LN/ELU overlaps the second half's matmuls.
            for half in ((0, 1), (2, 3)):
                for m in range(M):
                    mm_group(IC - 1, m, half)
                combine(IC - 1, half)
                epilogue(half)

    nc.compile()
    return nc


def _install_ntff_shim():
    """Provide antenv.axon_hooks (NTFF profiling hook) if the image lacks it.

    Mirrors the ctypes hook normally installed at boot: drives
    axon_{start,stop}_nrt_profile in libaxon_pjrt.so so run_bass_kernel_spmd
    trace=True can capture per-core NTFF profiles."""
    import sys
    import types
    import ctypes
    import contextlib

    try:
        from antenv.axon_hooks import get_axon_ntff_profile_hook  # noqa: F401
        return
    except ImportError:
        pass

    holder = {"hook": None}
    mod = types.ModuleType("antenv.axon_hooks")
    mod.set_axon_ntff_profile_hook = lambda h: holder.__setitem__("hook", h)
    mod.get_axon_ntff_profile_hook = lambda: holder["hook"]

    so_path = "/opt/axon/libaxon_pjrt.so"
    if os.path.exists(so_path):
        lib = ctypes.CDLL(so_path)
        if hasattr(lib, "axon_start_nrt_profile"):
            lib.axon_start_nrt_profile.argtypes = [
                ctypes.POINTER(ctypes.c_int64),
                ctypes.c_size_t,
            ]
            lib.axon_start_nrt_profile.restype = ctypes.c_int64
            lib.axon_stop_nrt_profile.argtypes = [ctypes.c_char_p]
            lib.axon_stop_nrt_profile.restype = ctypes.c_int64

            @contextlib.contextmanager
            def _hook(output_dir, device_ids):
                import jax

                jax.devices()
                if device_ids:
                    ids = (ctypes.c_int64 * len(device_ids))(*device_ids)
                    rc = lib.axon_start_nrt_profile(ids, len(device_ids))
                else:
                    rc = lib.axon_start_nrt_profile(None, 0)
                if rc != 0:
                    raise RuntimeError(f"axon_start_nrt_profile rc={rc}")
                try:
                    yield
                finally:
                    n = lib.axon_stop_nrt_profile(str(output_dir).encode())
                    print(f"profile: {n} file(s) written to {output_dir}")

            holder["hook"] = _hook

    sys.modules["antenv.axon_hooks"] = mod


def _prepare_inputs(x, Ws, bs, pW, pb, gW, gb):
    bf16 = ml_dtypes.bfloat16
    x = np.ascontiguousarray(np.asarray(x, np.float32))
    Ws = np.asarray(Ws, np.float32)
    bs = np.ascontiguousarray(np.asarray(bs, np.float32))
    pW = np.ascontiguousarray(np.asarray(pW, np.float32))
    pb = np.asarray(pb, np.float32)
    gW = np.ascontiguousarray(np.asarray(gW, np.float32))
    gb = np.ascontiguousarray(np.asarray(gb, np.float32))

    # W[(m, j), i] laid out as [ic, m, jj, jt, ii] so each (ic, m) block is a
    # contiguous 2 MB DMA with 16 KB per partition.
    w_host = np.ascontiguousarray(
        Ws.reshape(M, IC, 512, JT, 128).transpose(1, 0, 4, 3, 2)
    ).astype(bf16)

    in_maps = []
    for c in range(NCORES):
        rb = slice(c * BS, (c + 1) * BS)
        xc = x[rb]                                   # [512, 2048]
        xt = np.ascontiguousarray(
            xc.T.reshape(JT, 128, BS).transpose(1, 0, 2)
        ).astype(bf16)                               # [jj, jt, b]
        pbs_host = np.zeros((128, DO + BS), np.float32)
        pbs_host[:M, :DO] = bs
        pbs_host[:M, DO:] = pb[rb].T
        in_maps.append(
            {
                "w": w_host,
                "xt": xt,
                "pw": np.ascontiguousarray(pW[rb]),
                "pbs": pbs_host,
                "gw": np.ascontiguousarray(gW[rb]),
                "gb": np.ascontiguousarray(gb[rb]),
            }
        )
    return in_maps


def kernel(x, Ws, bs, pW, pb, gW, gb):
    global _PROGRAM, LAST_EXEC_NS, LAST_RESULTS
    from concourse.bass_utils import run_bass_kernel_spmd

    if os.environ.get("KERNEL_TRACE", "0") == "1":
        _install_ntff_shim()

    if _PROGRAM is None:
        _PROGRAM = _build_program()

    in_maps = _prepare_inputs(x, Ws, bs, pW, pb, gW, gb)
    trace = os.environ.get("KERNEL_TRACE", "0") == "1"
    res = run_bass_kernel_spmd(
        _PROGRAM, in_maps, core_ids=list(range(NCORES)), trace=trace
    )
    LAST_RESULTS = res
    LAST_EXEC_NS = res.exec_time_ns
    return np.concatenate([res.results[c]["out"] for c in range(NCORES)], axis=0)


# revision 9
# speedup vs baseline: 1.0090x; 1.0090x over previous
import os

import numpy as np
import ml_dtypes

# nn_GateModLinear: B=4096, M=8 experts, DI=DO=2048, LN eps=1e-5.
#   h[b,m,i] = sum_j Ws[m,i,j] x[b,j]
#   Wx = gW * sum_m pW[b,m] h[b,m,:]
#   z  = Wx + gb * (pb @ bs)
#   out = ELU(LayerNorm(z))
#
# Strategy: data-parallel over batch across 8 NeuronCores (512 rows each).
# Per core: one bf16 [512,16384]x[16384,2048] matmul on TensorE, streamed
# over W once; per-expert PSUM tiles are mixed with pW on ScalarE (per-
# partition scale) + VectorE adds; bias path is a K=128 zero-padded matmul;
# gating, LayerNorm and ELU run on VectorE/ScalarE. ELU uses
#   elu(y) = relu(y) + min(exp(y)-1, 0).
# The last output-column pass is split into two batch halves so half of the
# LN/ELU epilogue overlaps the tail of the matmul stream.

B, M, DI, DO = 4096, 8, 2048, 2048
NCORES = 8
BS = B // NCORES      # 512 batch rows per core
BT = BS // 128        # 4 b-tiles of 128 (partition dim)
IC = DO // 512        # 4 output chunks of 512
JT = DI // 128        # 16 contraction tiles of 128 per expert
LN_EPS = 1e-5

_PROGRAM = None          # cached bass.Bass program
LAST_EXEC_NS = None
LAST_RESULTS = None


def _build_program():
    import concourse.bacc as bacc
    import concourse.bass as bass
    import concourse.tile as tile
    import concourse.mybir as mybir

    f32 = mybir.dt.float32
    bf16 = mybir.dt.bfloat16
    AF = mybir.ActivationFunctionType
    OP = mybir.AluOpType

    nc = bacc.Bacc("TRN2")

    # [ic, m, jj, jt, ii]: W[(m, jt*128+jj), ic*512+ii] = Ws[m, i, j]
    w_d = nc.dram_tensor("w", [IC, M, 128, JT, 512], bf16, kind="ExternalInput")
    # [jj, jt, b]: xT for this core, k=jt*128+jj on partitions
    xt_d = nc.dram_tensor("xt", [128, JT, BS], bf16, kind="ExternalInput")
    pw_d = nc.dram_tensor("pw", [BS, M], f32, kind="ExternalInput")
    # [:, :DO] = bs zero-padded to 128 rows; [:, DO:] = pb.T zero-padded.
    pbs_d = nc.dram_tensor("pbs", [128, DO + BS], f32, kind="ExternalInput")
    gw_d = nc.dram_tensor("gw", [BS, DO], f32, kind="ExternalInput")
    gb_d = nc.dram_tensor("gb", [BS, DO], f32, kind="ExternalInput")
    out_d = nc.dram_tensor("out", [BS, DO], f32, kind="ExternalOutput")

    with tile.TileContext(nc) as tc:
        with (
            tc.tile_pool(name="const", bufs=1) as cpool,
            tc.tile_pool(name="wpool", bufs=3) as wpool,
            tc.tile_pool(name="acc", bufs=1) as apool,
            tc.tile_pool(name="mix", bufs=3) as mpool,
            tc.tile_pool(name="gio", bufs=4) as gpool,
            tc.tile_pool(name="epi", bufs=1) as epool,
            tc.tile_pool(name="psum", bufs=2, space=bass.MemorySpace.PSUM) as pspool,
        ):
            # ---- constants / persistent tiles ----
            # pbs first: the bias matmuls depend only on it, so PE starts
            # (and HAM-warms) as early as possible.
            pbs_s = cpool.tile([128, DO + BS], f32)
            nc.sync.dma_start(out=pbs_s[:], in_=pbs_d[:])
            xt_s = cpool.tile([128, JT, BS], bf16)
            nc.sync.dma_start(out=xt_s[:], in_=xt_d[:])
            pw_t = []
            for bt in range(BT):
                t = cpool.tile([128, M], f32, name=f"pw{bt}", tag=f"pw{bt}")
                nc.sync.dma_start(out=t[:], in_=pw_d[bt * 128:(bt + 1) * 128, :])
                pw_t.append(t)
            eps_t = cpool.tile([128, 1], f32, name="eps", tag="eps")
            nc.vector.memset(eps_t[:], LN_EPS)

            z_t = [apool.tile([128, DO], f32, name=f"z{bt}", tag=f"z{bt}")
                   for bt in range(BT)]
            wx_t = [apool.tile([128, DO], f32, name=f"wx{bt}", tag=f"wx{bt}")
                    for bt in range(BT)]
            st_t = [apool.tile([128, IC, 6], f32, name=f"st{bt}", tag=f"st{bt}")
                    for bt in range(BT)]

            # ---- phase A: z = gb * (pb @ bs) (bias path, K padded to 128) ----
            # gb loads ride the Scalar HWDGE ring so the Sync ring is free for
            # the W stream.
            for bt in range(BT):
                for ic in range(IC):
                    ps = pspool.tile([128, 512], f32, name=f"ps{bt}", tag=f"b{bt}")
                    nc.tensor.matmul(
                        ps[:],
                        pbs_s[:, DO + bt * 128:DO + (bt + 1) * 128],
                        pbs_s[:, ic * 512:(ic + 1) * 512],
                        start=True,
                        stop=True,
                    )
                    gbt = gpool.tile([128, 512], f32, name="gbt", tag="gb")
                    nc.scalar.dma_start(
                        out=gbt[:],
                        in_=gb_d[bt * 128:(bt + 1) * 128, ic * 512:(ic + 1) * 512],
                    )
                    nc.vector.tensor_mul(
                        z_t[bt][:, ic * 512:(ic + 1) * 512], gbt[:], ps[:]
                    )

            def mm_group(ic, m, bts):
                """One (ic, m) accumulation group over the given b-tiles."""
                w_s = wpool.tile([128, JT, 512], bf16, name="ws", tag="w")
                nc.sync.dma_start(out=w_s[:], in_=w_d[ic, m])
                pss = {
                    bt: pspool.tile([128, 512], f32, name=f"ps{bt}", tag=f"b{bt}")
                    for bt in bts
                }
                for jt in range(JT):
                    for bt in bts:
                        nc.tensor.matmul(
                            pss[bt][:],
                            xt_s[:, jt, bt * 128:(bt + 1) * 128],
                            w_s[:, jt, :],
                            start=(jt == 0),
                            stop=(jt == JT - 1),
                        )
                for bt in bts:
                    wxs = wx_t[bt][:, ic * 512:(ic + 1) * 512]
                    if m == 0:
                        nc.scalar.activation(
                            wxs, pss[bt][:], AF.Copy, scale=pw_t[bt][:, 0:1]
                        )
                    else:
                        tmp = mpool.tile([128, 512], f32, name="tmp", tag="tmp")
                        nc.scalar.activation(
                            tmp[:], pss[bt][:], AF.Copy, scale=pw_t[bt][:, m:m + 1]
                        )
                        nc.vector.tensor_add(wxs, wxs, tmp[:])

            def combine(ic, bts):
                """z[:, ic] += gW * wx and bn_stats for the chunk."""
                for bt in bts:
                    gwt = gpool.tile([128, 512], f32, name="gwt", tag="gw")
                    nc.scalar.dma_start(
                        out=gwt[:],
                        in_=gw_d[bt * 128:(bt + 1) * 128, ic * 512:(ic + 1) * 512],
                    )
                    wxs = wx_t[bt][:, ic * 512:(ic + 1) * 512]
                    zs = z_t[bt][:, ic * 512:(ic + 1) * 512]
                    tmp2 = mpool.tile([128, 512], f32, name="tmp2", tag="tmp2")
                    nc.vector.tensor_mul(tmp2[:], gwt[:], wxs)
                    nc.vector.tensor_add(zs, zs, tmp2[:])
                    nc.vector.bn_stats(st_t[bt][:, ic, :], zs)

            def epilogue(bts):
                """LayerNorm + ELU + store for the given b-tiles."""
                mvs, rstds = {}, {}
                for bt in bts:
                    mv = mpool.tile([128, 2], f32, name="mv", tag=f"mv{bt}")
                    nc.vector.bn_aggr(mv[:], st_t[bt][:])
                    mvs[bt] = mv
                for bt in bts:  # grouped: one ACT Sqrt table load
                    sd = mpool.tile([128, 1], f32, name="sd", tag=f"sd{bt}")
                    nc.scalar.activation(sd[:], mvs[bt][:, 1:2], AF.Sqrt,
                                         bias=eps_t[:])
                    rstds[bt] = sd
                for bt in bts:
                    nc.vector.reciprocal(rstds[bt][:], rstds[bt][:])
                ys, es = {}, {}
                for bt in bts:
                    y = epool.tile([128, DO], f32, name="y", tag=f"y{bt % 2}")
                    nc.vector.tensor_scalar(
                        out=y[:],
                        in0=z_t[bt][:],
                        scalar1=mvs[bt][:, 0:1],
                        scalar2=rstds[bt][:],
                        op0=OP.subtract,
                        op1=OP.mult,
                    )
                    ys[bt] = y
                for bt in bts:  # grouped: one ACT Exp table load
                    e = epool.tile([128, DO], f32, name="e", tag=f"e{bt % 2}")
                    nc.scalar.activation(e[:], ys[bt][:], AF.Exp)
                    es[bt] = e
                for bt in bts:
                    y, e = ys[bt], es[bt]
                    nc.vector.tensor_scalar_max(y[:], y[:], 0.0)
                    nc.vector.tensor_scalar(
                        out=e[:],
                        in0=e[:],
                        scalar1=1.0,
                        scalar2=1.0,
                        op0=OP.min,
                        op1=OP.subtract,
                    )
                    nc.vector.tensor_add(y[:], y[:], e[:])
                    nc.scalar.dma_start(
                        out=out_d[bt * 128:(bt + 1) * 128, :], in_=y[:]
                    )

            # ---- phase B: main matmul + expert mix + gate + stats ----
            for ic in range(IC):
                for m in range(M):
                    mm_group(ic, m, range(BT))
                combine(ic, range(BT))
            # ---- phase C: LN + ELU in b-tile pairs (grouped ACT funcs) ----
            for half in ((0, 1), (2, 3)):
                epilogue(half)

    nc.compile()
    return nc


def _install_ntff_shim():
    """Provide antenv.axon_hooks (NTFF profiling hook) if the image lacks it.

    Mirrors the ctypes hook normally installed at boot: drives
    axon_{start,stop}_nrt_profile in libaxon_pjrt.so so run_bass_kernel_spmd
    trace=True can capture per-core NTFF profiles."""
    import sys
    import types
    import ctypes
    import contextlib

    try:
        from antenv.axon_hooks import get_axon_ntff_profile_hook  # noqa: F401
        return
    except ImportError:
        pass

    holder = {"hook": None}
    mod = types.ModuleType("antenv.axon_hooks")
    mod.set_axon_ntff_profile_hook = lambda h: holder.__setitem__("hook", h)
    mod.get_axon_ntff_profile_hook = lambda: holder["hook"]

    so_path = "/opt/axon/libaxon_pjrt.so"
    if os.path.exists(so_path):
        lib = ctypes.CDLL(so_path)
        if hasattr(lib, "axon_start_nrt_profile"):
            lib.axon_start_nrt_profile.argtypes = [
                ctypes.POINTER(ctypes.c_int64),
                ctypes.c_size_t,
            ]
            lib.axon_start_nrt_profile.restype = ctypes.c_int64
            lib.axon_stop_nrt_profile.argtypes = [ctypes.c_char_p]
            lib.axon_stop_nrt_profile.restype = ctypes.c_int64

            @contextlib.contextmanager
            def _hook(output_dir, device_ids):
                import jax

                jax.devices()
                if device_ids:
                    ids = (ctypes.c_int64 * len(device_ids))(*device_ids)
                    rc = lib.axon_start_nrt_profile(ids, len(device_ids))
                else:
                    rc = lib.axon_start_nrt_profile(None, 0)
                if rc != 0:
                    raise RuntimeError(f"axon_start_nrt_profile rc={rc}")
                try:
                    yield
                finally:
                    n = lib.axon_stop_nrt_profile(str(output_dir).encode())
                    print(f"profile: {n} file(s) written to {output_dir}")

            holder["hook"] = _hook

    sys.modules["antenv.axon_hooks"] = mod


def _prepare_inputs(x, Ws, bs, pW, pb, gW, gb):
    bf16 = ml_dtypes.bfloat16
    x = np.ascontiguousarray(np.asarray(x, np.float32))
    Ws = np.asarray(Ws, np.float32)
    bs = np.ascontiguousarray(np.asarray(bs, np.float32))
    pW = np.ascontiguousarray(np.asarray(pW, np.float32))
    pb = np.asarray(pb, np.float32)
    gW = np.ascontiguousarray(np.asarray(gW, np.float32))
    gb = np.ascontiguousarray(np.asarray(gb, np.float32))

    # W[(m, j), i] laid out as [ic, m, jj, jt, ii] so each (ic, m) block is a
    # contiguous 2 MB DMA with 16 KB per partition.
    w_host = np.ascontiguousarray(
        Ws.reshape(M, IC, 512, JT, 128).transpose(1, 0, 4, 3, 2)
    ).astype(bf16)

    in_maps = []
    for c in range(NCORES):
        rb = slice(c * BS, (c + 1) * BS)
        xc = x[rb]                                   # [512, 2048]
        xt = np.ascontiguousarray(
            xc.T.reshape(JT, 128, BS).transpose(1, 0, 2)
        ).astype(bf16)                               # [jj, jt, b]
        pbs_host = np.zeros((128, DO + BS), np.float32)
        pbs_host[:M, :DO] = bs
        pbs_host[:M, DO:] = pb[rb].T
        in_maps.append(
            {
                "w": w_host,
                "xt": xt,
                "pw": np.ascontiguousarray(pW[rb]),
                "pbs": pbs_host,
                "gw": np.ascontiguousarray(gW[rb]),
                "gb": np.ascontiguousarray(gb[rb]),
            }
        )
    return in_maps


def kernel(x, Ws, bs, pW, pb, gW, gb):
    global _PROGRAM, LAST_EXEC_NS, LAST_RESULTS
    from concourse.bass_utils import run_bass_kernel_spmd

    if os.environ.get("KERNEL_TRACE", "0") == "1":
        _install_ntff_shim()

    if _PROGRAM is None:
        _PROGRAM = _build_program()

    in_maps = _prepare_inputs(x, Ws, bs, pW, pb, gW, gb)
    trace = os.environ.get("KERNEL_TRACE", "0") == "1"
    res = run_bass_kernel_spmd(
        _PROGRAM, in_maps, core_ids=list(range(NCORES)), trace=trace
    )
    LAST_RESULTS = res
    LAST_EXEC_NS = res.exec_time_ns
    return np.concatenate([res.results[c]["out"] for c in range(NCORES)], axis=0)


# revision 10
# speedup vs baseline: 1.0240x; 1.0149x over previous
import os

import numpy as np
import ml_dtypes

# nn_GateModLinear: B=4096, M=8 experts, DI=DO=2048, LN eps=1e-5.
#   h[b,m,i] = sum_j Ws[m,i,j] x[b,j]
#   Wx = gW * sum_m pW[b,m] h[b,m,:]
#   z  = Wx + gb * (pb @ bs)
#   out = ELU(LayerNorm(z))
#
# Strategy: data-parallel over batch across 8 NeuronCores (512 rows each).
# Per core: one bf16 [512,16384]x[16384,2048] matmul on TensorE, streamed
# over W once; per-expert PSUM tiles are mixed with pW on ScalarE (per-
# partition scale) + VectorE adds; bias path is a K=128 zero-padded matmul;
# gating, LayerNorm and ELU run on VectorE/ScalarE. ELU uses
#   elu(y) = relu(y) + min(exp(y)-1, 0).
# The last output-column pass is split into two batch halves so half of the
# LN/ELU epilogue overlaps the tail of the matmul stream.

B, M, DI, DO = 4096, 8, 2048, 2048
NCORES = 8
BS = B // NCORES      # 512 batch rows per core
BT = BS // 128        # 4 b-tiles of 128 (partition dim)
IC = DO // 512        # 4 output chunks of 512
JT = DI // 128        # 16 contraction tiles of 128 per expert
LN_EPS = 1e-5

_PROGRAM = None          # cached bass.Bass program
LAST_EXEC_NS = None
LAST_RESULTS = None


def _build_program():
    import concourse.bacc as bacc
    import concourse.bass as bass
    import concourse.tile as tile
    import concourse.mybir as mybir

    f32 = mybir.dt.float32
    bf16 = mybir.dt.bfloat16
    AF = mybir.ActivationFunctionType
    OP = mybir.AluOpType

    nc = bacc.Bacc("TRN2")

    # [ic, m, jj, jt, ii]: W[(m, jt*128+jj), ic*512+ii] = Ws[m, i, j]
    w_d = nc.dram_tensor("w", [IC, M, 128, JT, 512], bf16, kind="ExternalInput")
    # [jj, jt, b]: xT for this core, k=jt*128+jj on partitions
    xt_d = nc.dram_tensor("xt", [128, JT, BS], bf16, kind="ExternalInput")
    pw_d = nc.dram_tensor("pw", [BS, M], f32, kind="ExternalInput")
    # [:, :DO] = bs zero-padded to 128 rows; [:, DO:] = pb.T zero-padded.
    pbs_d = nc.dram_tensor("pbs", [128, DO + BS], f32, kind="ExternalInput")
    gw_d = nc.dram_tensor("gw", [BS, DO], f32, kind="ExternalInput")
    gb_d = nc.dram_tensor("gb", [BS, DO], f32, kind="ExternalInput")
    out_d = nc.dram_tensor("out", [BS, DO], f32, kind="ExternalOutput")

    with tile.TileContext(nc) as tc:
        with (
            tc.tile_pool(name="const", bufs=1) as cpool,
            tc.tile_pool(name="wpool", bufs=3) as wpool,
            tc.tile_pool(name="acc", bufs=1) as apool,
            tc.tile_pool(name="mix", bufs=3) as mpool,
            tc.tile_pool(name="gio", bufs=4) as gpool,
            tc.tile_pool(name="epi", bufs=1) as epool,
            tc.tile_pool(name="psum", bufs=2, space=bass.MemorySpace.PSUM) as pspool,
        ):
            # ---- constants / persistent tiles ----
            # pbs first: the bias matmuls depend only on it, so PE starts
            # (and HAM-warms) as early as possible.
            pbs_s = cpool.tile([128, DO + BS], f32)
            nc.sync.dma_start(out=pbs_s[:], in_=pbs_d[:])
            xt_s = cpool.tile([128, JT, BS], bf16)
            nc.scalar.dma_start(out=xt_s[:], in_=xt_d[:])
            pw_t = []
            for bt in range(BT):
                t = cpool.tile([128, M], f32, name=f"pw{bt}", tag=f"pw{bt}")
                nc.sync.dma_start(out=t[:], in_=pw_d[bt * 128:(bt + 1) * 128, :])
                pw_t.append(t)
            eps_t = cpool.tile([128, 1], f32, name="eps", tag="eps")
            nc.vector.memset(eps_t[:], LN_EPS)

            z_t = [apool.tile([128, DO], f32, name=f"z{bt}", tag=f"z{bt}")
                   for bt in range(BT)]
            wx_t = [apool.tile([128, DO], f32, name=f"wx{bt}", tag=f"wx{bt}")
                    for bt in range(BT)]
            st_t = [apool.tile([128, IC, 6], f32, name=f"st{bt}", tag=f"st{bt}")
                    for bt in range(BT)]

            # ---- phase A: z = gb * (pb @ bs) (bias path, K padded to 128) ----
            # gb loads ride the Scalar HWDGE ring so the Sync ring is free for
            # the W stream.
            for bt in range(BT):
                for ic in range(IC):
                    ps = pspool.tile([128, 512], f32, name=f"ps{bt}", tag=f"b{bt}")
                    nc.tensor.matmul(
                        ps[:],
                        pbs_s[:, DO + bt * 128:DO + (bt + 1) * 128],
                        pbs_s[:, ic * 512:(ic + 1) * 512],
                        start=True,
                        stop=True,
                    )
                    gbt = gpool.tile([128, 512], f32, name="gbt", tag="gb")
                    nc.scalar.dma_start(
                        out=gbt[:],
                        in_=gb_d[bt * 128:(bt + 1) * 128, ic * 512:(ic + 1) * 512],
                    )
                    nc.vector.tensor_mul(
                        z_t[bt][:, ic * 512:(ic + 1) * 512], gbt[:], ps[:]
                    )

            def mix(ic, m, bt, ps):
                wxs = wx_t[bt][:, ic * 512:(ic + 1) * 512]
                if m == 0:
                    nc.scalar.activation(
                        wxs, ps[:], AF.Copy, scale=pw_t[bt][:, 0:1]
                    )
                else:
                    tmp = mpool.tile([128, 512], f32, name="tmp", tag="tmp")
                    nc.scalar.activation(
                        tmp[:], ps[:], AF.Copy, scale=pw_t[bt][:, m:m + 1]
                    )
                    nc.vector.tensor_add(wxs, wxs, tmp[:])

            def mm_group(ic, m, bts, bt_outer=False):
                """One (ic, m) accumulation group over the given b-tiles."""
                w_s = wpool.tile([128, JT, 512], bf16, name="ws", tag="w")
                nc.sync.dma_start(out=w_s[:], in_=w_d[ic, m])
                pss = {
                    bt: pspool.tile([128, 512], f32, name=f"ps{bt}", tag=f"b{bt}")
                    for bt in bts
                }
                if bt_outer:
                    # serialize per b-tile so early tiles close (and mix)
                    # while later tiles are still on the PE
                    for bt in bts:
                        for jt in range(JT):
                            nc.tensor.matmul(
                                pss[bt][:],
                                xt_s[:, jt, bt * 128:(bt + 1) * 128],
                                w_s[:, jt, :],
                                start=(jt == 0),
                                stop=(jt == JT - 1),
                            )
                        mix(ic, m, bt, pss[bt])
                    return
                for jt in range(JT):
                    for bt in bts:
                        nc.tensor.matmul(
                            pss[bt][:],
                            xt_s[:, jt, bt * 128:(bt + 1) * 128],
                            w_s[:, jt, :],
                            start=(jt == 0),
                            stop=(jt == JT - 1),
                        )
                for bt in bts:
                    mix(ic, m, bt, pss[bt])

            def combine(ic, bts):
                """z[:, ic] += gW * wx and bn_stats for the chunk."""
                for bt in bts:
                    gwt = gpool.tile([128, 512], f32, name="gwt", tag="gw")
                    nc.scalar.dma_start(
                        out=gwt[:],
                        in_=gw_d[bt * 128:(bt + 1) * 128, ic * 512:(ic + 1) * 512],
                    )
                    wxs = wx_t[bt][:, ic * 512:(ic + 1) * 512]
                    zs = z_t[bt][:, ic * 512:(ic + 1) * 512]
                    tmp2 = mpool.tile([128, 512], f32, name="tmp2", tag="tmp2")
                    nc.vector.tensor_mul(tmp2[:], gwt[:], wxs)
                    nc.vector.tensor_add(zs, zs, tmp2[:])
                    nc.vector.bn_stats(st_t[bt][:, ic, :], zs)

            def epilogue(bts):
                """LayerNorm + ELU + store for the given b-tiles."""
                mvs, rstds = {}, {}
                for bt in bts:
                    mv = mpool.tile([128, 2], f32, name="mv", tag=f"mv{bt}")
                    nc.vector.bn_aggr(mv[:], st_t[bt][:])
                    mvs[bt] = mv
                for bt in bts:  # grouped: one ACT Sqrt table load
                    sd = mpool.tile([128, 1], f32, name="sd", tag=f"sd{bt}")
                    nc.scalar.activation(sd[:], mvs[bt][:, 1:2], AF.Sqrt,
                                         bias=eps_t[:])
                    rstds[bt] = sd
                for bt in bts:
                    nc.vector.reciprocal(rstds[bt][:], rstds[bt][:])
                ys, es = {}, {}
                for bt in bts:
                    y = epool.tile([128, DO], f32, name="y", tag=f"y{bt % 2}")
                    nc.vector.tensor_scalar(
                        out=y[:],
                        in0=z_t[bt][:],
                        scalar1=mvs[bt][:, 0:1],
                        scalar2=rstds[bt][:],
                        op0=OP.subtract,
                        op1=OP.mult,
                    )
                    ys[bt] = y
                for bt in bts:  # grouped: one ACT Exp table load
                    e = epool.tile([128, DO], f32, name="e", tag=f"e{bt % 2}")
                    nc.scalar.activation(e[:], ys[bt][:], AF.Exp)
                    es[bt] = e
                for bt in bts:
                    y, e = ys[bt], es[bt]
                    nc.vector.tensor_scalar_max(y[:], y[:], 0.0)
                    nc.vector.tensor_scalar(
                        out=e[:],
                        in0=e[:],
                        scalar1=1.0,
                        scalar2=1.0,
                        op0=OP.min,
                        op1=OP.subtract,
                    )
                    nc.vector.tensor_add(y[:], y[:], e[:])
                    nc.scalar.dma_start(
                        out=out_d[bt * 128:(bt + 1) * 128, :], in_=y[:]
                    )

            # ---- phase B: main matmul + expert mix + gate + stats ----
            for ic in range(IC):
                last = ic == IC - 1
                for m in range(M - 1 if last else M):
                    mm_group(ic, m, range(BT))
                if not last:
                    combine(ic, range(BT))
            # Final expert group runs b-tile-serial so each pair's gate/LN/ELU
            # overlaps the remaining matmuls.
            for half in ((0, 1), (2, 3)):
                mm_group(IC - 1, M - 1, half, bt_outer=True)
                combine(IC - 1, half)
                epilogue(half)

    nc.compile()
    return nc


def _install_ntff_shim():
    """Provide antenv.axon_hooks (NTFF profiling hook) if the image lacks it.

    Mirrors the ctypes hook normally installed at boot: drives
    axon_{start,stop}_nrt_profile in libaxon_pjrt.so so run_bass_kernel_spmd
    trace=True can capture per-core NTFF profiles."""
    import sys
    import types
    import ctypes
    import contextlib

    try:
        from antenv.axon_hooks import get_axon_ntff_profile_hook  # noqa: F401
        return
    except ImportError:
        pass

    holder = {"hook": None}
    mod = types.ModuleType("antenv.axon_hooks")
    mod.set_axon_ntff_profile_hook = lambda h: holder.__setitem__("hook", h)
    mod.get_axon_ntff_profile_hook = lambda: holder["hook"]

    so_path = "/opt/axon/libaxon_pjrt.so"
    if os.path.exists(so_path):
        lib = ctypes.CDLL(so_path)
        if hasattr(lib, "axon_start_nrt_profile"):
            lib.axon_start_nrt_profile.argtypes = [
                ctypes.POINTER(ctypes.c_int64),
                ctypes.c_size_t,
            ]
            lib.axon_start_nrt_profile.restype = ctypes.c_int64
            lib.axon_stop_nrt_profile.argtypes = [ctypes.c_char_p]
            lib.axon_stop_nrt_profile.restype = ctypes.c_int64

            @contextlib.contextmanager
            def _hook(output_dir, device_ids):
                import jax

                jax.devices()
                if device_ids:
                    ids = (ctypes.c_int64 * len(device_ids))(*device_ids)
                    rc = lib.axon_start_nrt_profile(ids, len(device_ids))
                else:
                    rc = lib.axon_start_nrt_profile(None, 0)
                if rc != 0:
                    raise RuntimeError(f"axon_start_nrt_profile rc={rc}")
                try:
                    yield
                finally:
                    n = lib.axon_stop_nrt_profile(str(output_dir).encode())
                    print(f"profile: {n} file(s) written to {output_dir}")

            holder["hook"] = _hook

    sys.modules["antenv.axon_hooks"] = mod


def _prepare_inputs(x, Ws, bs, pW, pb, gW, gb):
    bf16 = ml_dtypes.bfloat16
    x = np.ascontiguousarray(np.asarray(x, np.float32))
    Ws = np.asarray(Ws, np.float32)
    bs = np.ascontiguousarray(np.asarray(bs, np.float32))
    pW = np.ascontiguousarray(np.asarray(pW, np.float32))
    pb = np.asarray(pb, np.float32)
    gW = np.ascontiguousarray(np.asarray(gW, np.float32))
    gb = np.ascontiguousarray(np.asarray(gb, np.float32))

    # W[(m, j), i] laid out as [ic, m, jj, jt, ii] so each (ic, m) block is a
    # contiguous 2 MB DMA with 16 KB per partition.
    w_host = np.ascontiguousarray(
        Ws.reshape(M, IC, 512, JT, 128).transpose(1, 0, 4, 3, 2)
    ).astype(bf16)

    in_maps = []
    for c in range(NCORES):
        rb = slice(c * BS, (c + 1) * BS)
        xc = x[rb]                                   # [512, 2048]
        xt = np.ascontiguousarray(
            xc.T.reshape(JT, 128, BS).transpose(1, 0, 2)
        ).astype(bf16)                               # [jj, jt, b]
        pbs_host = np.zeros((128, DO + BS), np.float32)
        pbs_host[:M, :DO] = bs
        pbs_host[:M, DO:] = pb[rb].T
        in_maps.append(
            {
                "w": w_host,
                "xt": xt,
                "pw": np.ascontiguousarray(pW[rb]),
                "pbs": pbs_host,
                "gw": np.ascontiguousarray(gW[rb]),
                "gb": np.ascontiguousarray(gb[rb]),
            }
        )
    return in_maps


def kernel(x, Ws, bs, pW, pb, gW, gb):
    global _PROGRAM, LAST_EXEC_NS, LAST_RESULTS
    from concourse.bass_utils import run_bass_kernel_spmd

    if os.environ.get("KERNEL_TRACE", "0") == "1":
        _install_ntff_shim()

    if _PROGRAM is None:
        _PROGRAM = _build_program()

    in_maps = _prepare_inputs(x, Ws, bs, pW, pb, gW, gb)
    trace = os.environ.get("KERNEL_TRACE", "0") == "1"
    res = run_bass_kernel_spmd(
        _PROGRAM, in_maps, core_ids=list(range(NCORES)), trace=trace
    )
    LAST_RESULTS = res
    LAST_EXEC_NS = res.exec_time_ns
    return np.concatenate([res.results[c]["out"] for c in range(NCORES)], axis=0)


# revision 11
# speedup vs baseline: 1.0466x; 1.0221x over previous
import os

import numpy as np
import ml_dtypes

# nn_GateModLinear: B=4096, M=8 experts, DI=DO=2048, LN eps=1e-5.
#   h[b,m,i] = sum_j Ws[m,i,j] x[b,j]
#   Wx = gW * sum_m pW[b,m] h[b,m,:]
#   z  = Wx + gb * (pb @ bs)
#   out = ELU(LayerNorm(z))
#
# Strategy: data-parallel over batch across 8 NeuronCores (512 rows each).
# Per core: one bf16 [512,16384]x[16384,2048] matmul on TensorE, streamed
# over W once; per-expert PSUM tiles are mixed with pW on ScalarE (per-
# partition scale) + VectorE adds; bias path is a K=128 zero-padded matmul;
# gating, LayerNorm and ELU run on VectorE/ScalarE. ELU uses
#   elu(y) = relu(y) + min(exp(y)-1, 0).
# The last output-column pass is split into two batch halves so half of the
# LN/ELU epilogue overlaps the tail of the matmul stream.

B, M, DI, DO = 4096, 8, 2048, 2048
NCORES = 8
BS = B // NCORES      # 512 batch rows per core
BT = BS // 128        # 4 b-tiles of 128 (partition dim)
IC = DO // 512        # 4 output chunks of 512
JT = DI // 128        # 16 contraction tiles of 128 per expert
LN_EPS = 1e-5

_PROGRAM = None          # cached bass.Bass program
LAST_EXEC_NS = None
LAST_RESULTS = None


def _build_program():
    import concourse.bacc as bacc
    import concourse.bass as bass
    import concourse.tile as tile
    import concourse.mybir as mybir

    f32 = mybir.dt.float32
    bf16 = mybir.dt.bfloat16
    AF = mybir.ActivationFunctionType
    OP = mybir.AluOpType

    nc = bacc.Bacc("TRN2")

    # [ic, m, jj, jt, ii]: W[(m, jt*128+jj), ic*512+ii] = Ws[m, i, j]
    w_d = nc.dram_tensor("w", [IC, M, 128, JT, 512], bf16, kind="ExternalInput")
    # [jj, jt, b]: xT for this core, k=jt*128+jj on partitions
    xt_d = nc.dram_tensor("xt", [128, JT, BS], bf16, kind="ExternalInput")
    pw_d = nc.dram_tensor("pw", [BS, M], f32, kind="ExternalInput")
    # [:, :DO] = bs zero-padded to 128 rows; [:, DO:] = pb.T zero-padded.
    pbs_d = nc.dram_tensor("pbs", [128, DO + BS], f32, kind="ExternalInput")
    gw_d = nc.dram_tensor("gw", [BS, DO], f32, kind="ExternalInput")
    gb_d = nc.dram_tensor("gb", [BS, DO], f32, kind="ExternalInput")
    out_d = nc.dram_tensor("out", [BS, DO], f32, kind="ExternalOutput")

    with tile.TileContext(nc) as tc:
        with (
            tc.tile_pool(name="const", bufs=1) as cpool,
            tc.tile_pool(name="wpool", bufs=3) as wpool,
            tc.tile_pool(name="acc", bufs=1) as apool,
            tc.tile_pool(name="mix", bufs=3) as mpool,
            tc.tile_pool(name="gio", bufs=4) as gpool,
            tc.tile_pool(name="epi", bufs=1) as epool,
            tc.tile_pool(name="psum", bufs=2, space=bass.MemorySpace.PSUM) as pspool,
        ):
            # ---- constants / persistent tiles ----
            # pbs first: the bias matmuls depend only on it, so PE starts
            # (and HAM-warms) as early as possible.
            pbs_s = cpool.tile([128, DO + BS], f32)
            nc.sync.dma_start(out=pbs_s[:, DO:], in_=pbs_d[:, DO:])
            for c0 in range(0, DO, 512):
                nc.sync.dma_start(out=pbs_s[:, c0:c0 + 512],
                                  in_=pbs_d[:, c0:c0 + 512])
            xt_s = cpool.tile([128, JT, BS], bf16)
            nc.scalar.dma_start(out=xt_s[:], in_=xt_d[:])
            pw_t = []
            for bt in range(BT):
                t = cpool.tile([128, M], f32, name=f"pw{bt}", tag=f"pw{bt}")
                nc.sync.dma_start(out=t[:], in_=pw_d[bt * 128:(bt + 1) * 128, :])
                pw_t.append(t)
            eps_t = cpool.tile([128, 1], f32, name="eps", tag="eps")
            nc.vector.memset(eps_t[:], LN_EPS)

            z_t = [apool.tile([128, DO], f32, name=f"z{bt}", tag=f"z{bt}")
                   for bt in range(BT)]
            wx_t = [apool.tile([128, DO], f32, name=f"wx{bt}", tag=f"wx{bt}")
                    for bt in range(BT)]
            st_t = [apool.tile([128, IC, 6], f32, name=f"st{bt}", tag=f"st{bt}")
                    for bt in range(BT)]

            # ---- phase A: z = gb * (pb @ bs) (bias path, K padded to 128) ----
            # gb loads ride the Scalar HWDGE ring so the Sync ring is free for
            # the W stream.
            for bt in range(BT):
                for ic in range(IC):
                    ps = pspool.tile([128, 512], f32, name=f"ps{bt}", tag=f"b{bt}")
                    nc.tensor.matmul(
                        ps[:],
                        pbs_s[:, DO + bt * 128:DO + (bt + 1) * 128],
                        pbs_s[:, ic * 512:(ic + 1) * 512],
                        start=True,
                        stop=True,
                    )
                    gbt = gpool.tile([128, 512], f32, name="gbt", tag="gb")
                    nc.scalar.dma_start(
                        out=gbt[:],
                        in_=gb_d[bt * 128:(bt + 1) * 128, ic * 512:(ic + 1) * 512],
                    )
                    nc.vector.tensor_mul(
                        z_t[bt][:, ic * 512:(ic + 1) * 512], gbt[:], ps[:]
                    )

            def mix(ic, m, bt, ps):
                wxs = wx_t[bt][:, ic * 512:(ic + 1) * 512]
                if m == 0:
                    nc.scalar.activation(
                        wxs, ps[:], AF.Copy, scale=pw_t[bt][:, 0:1]
                    )
                else:
                    tmp = mpool.tile([128, 512], f32, name="tmp", tag="tmp")
                    nc.scalar.activation(
                        tmp[:], ps[:], AF.Copy, scale=pw_t[bt][:, m:m + 1]
                    )
                    nc.vector.tensor_add(wxs, wxs, tmp[:])

            def load_w(ic, m):
                w_s = wpool.tile([128, JT, 512], bf16, name="ws", tag="w")
                for j0 in range(0, JT, 4):
                    nc.sync.dma_start(out=w_s[:, j0:j0 + 4, :],
                                      in_=w_d[ic, m, :, j0:j0 + 4, :])
                return w_s

            def mm_group(ic, m, bts, bt_outer=False, w_s=None):
                """One (ic, m) accumulation group over the given b-tiles."""
                if w_s is None:
                    w_s = load_w(ic, m)
                pss = {
                    bt: pspool.tile([128, 512], f32, name=f"ps{bt}", tag=f"b{bt}")
                    for bt in bts
                }
                if bt_outer:
                    # serialize per b-tile so early tiles close (and mix)
                    # while later tiles are still on the PE
                    for bt in bts:
                        for jt in range(JT):
                            nc.tensor.matmul(
                                pss[bt][:],
                                xt_s[:, jt, bt * 128:(bt + 1) * 128],
                                w_s[:, jt, :],
                                start=(jt == 0),
                                stop=(jt == JT - 1),
                            )
                        mix(ic, m, bt, pss[bt])
                    return
                for jt in range(JT):
                    for bt in bts:
                        nc.tensor.matmul(
                            pss[bt][:],
                            xt_s[:, jt, bt * 128:(bt + 1) * 128],
                            w_s[:, jt, :],
                            start=(jt == 0),
                            stop=(jt == JT - 1),
                        )
                for bt in bts:
                    mix(ic, m, bt, pss[bt])

            def combine(ic, bts):
                """z[:, ic] += gW * wx and bn_stats for the chunk."""
                for bt in bts:
                    gwt = gpool.tile([128, 512], f32, name="gwt", tag="gw")
                    nc.scalar.dma_start(
                        out=gwt[:],
                        in_=gw_d[bt * 128:(bt + 1) * 128, ic * 512:(ic + 1) * 512],
                    )
                    wxs = wx_t[bt][:, ic * 512:(ic + 1) * 512]
                    zs = z_t[bt][:, ic * 512:(ic + 1) * 512]
                    tmp2 = mpool.tile([128, 512], f32, name="tmp2", tag="tmp2")
                    nc.vector.tensor_mul(tmp2[:], gwt[:], wxs)
                    nc.vector.tensor_add(zs, zs, tmp2[:])
                    nc.vector.bn_stats(st_t[bt][:, ic, :], zs)

            def epilogue(bts):
                """LayerNorm + ELU + store for the given b-tiles."""
                mvs, rstds = {}, {}
                for bt in bts:
                    mv = mpool.tile([128, 2], f32, name="mv", tag=f"mv{bt}")
                    nc.vector.bn_aggr(mv[:], st_t[bt][:])
                    mvs[bt] = mv
                for bt in bts:  # grouped: one ACT Sqrt table load
                    sd = mpool.tile([128, 1], f32, name="sd", tag=f"sd{bt}")
                    nc.scalar.activation(sd[:], mvs[bt][:, 1:2], AF.Sqrt,
                                         bias=eps_t[:])
                    rstds[bt] = sd
                for bt in bts:
                    nc.vector.reciprocal(rstds[bt][:], rstds[bt][:])
                ys, es = {}, {}
                for bt in bts:
                    y = epool.tile([128, DO], f32, name="y", tag=f"y{bt % 2}")
                    nc.vector.tensor_scalar(
                        out=y[:],
                        in0=z_t[bt][:],
                        scalar1=mvs[bt][:, 0:1],
                        scalar2=rstds[bt][:],
                        op0=OP.subtract,
                        op1=OP.mult,
                    )
                    ys[bt] = y
                for bt in bts:  # grouped: one ACT Exp table load
                    e = epool.tile([128, DO], f32, name="e", tag=f"e{bt % 2}")
                    nc.scalar.activation(e[:], ys[bt][:], AF.Exp)
                    es[bt] = e
                for bt in bts:
                    y, e = ys[bt], es[bt]
                    nc.vector.tensor_scalar_max(y[:], y[:], 0.0)
                    nc.vector.tensor_scalar(
                        out=e[:],
                        in0=e[:],
                        scalar1=1.0,
                        scalar2=1.0,
                        op0=OP.min,
                        op1=OP.subtract,
                    )
                    nc.vector.tensor_add(y[:], y[:], e[:])
                    nc.scalar.dma_start(
                        out=out_d[bt * 128:(bt + 1) * 128, :], in_=y[:]
                    )

            # ---- phase B: main matmul + expert mix + gate + stats ----
            for ic in range(IC):
                last = ic == IC - 1
                for m in range(M - 1 if last else M):
                    mm_group(ic, m, range(BT))
                if not last:
                    combine(ic, range(BT))
            # Final expert group runs b-tile-serial so each pair's gate/LN/ELU
            # overlaps the remaining matmuls.
            w_last = load_w(IC - 1, M - 1)
            for half in ((0, 1), (2, 3)):
                mm_group(IC - 1, M - 1, half, bt_outer=True, w_s=w_last)
                combine(IC - 1, half)
                epilogue(half)

    nc.compile()
    return nc


def _install_ntff_shim():
    """Provide antenv.axon_hooks (NTFF profiling hook) if the image lacks it.

    Mirrors the ctypes hook normally installed at boot: drives
    axon_{start,stop}_nrt_profile in libaxon_pjrt.so so run_bass_kernel_spmd
    trace=True can capture per-core NTFF profiles."""
    import sys
    import types
    import ctypes
    import contextlib

    try:
        from antenv.axon_hooks import get_axon_ntff_profile_hook  # noqa: F401
        return
    except ImportError:
        pass

    holder = {"hook": None}
    mod = types.ModuleType("antenv.axon_hooks")
    mod.set_axon_ntff_profile_hook = lambda h: holder.__setitem__("hook", h)
    mod.get_axon_ntff_profile_hook = lambda: holder["hook"]

    so_path = "/opt/axon/libaxon_pjrt.so"
    if os.path.exists(so_path):
        lib = ctypes.CDLL(so_path)
        if hasattr(lib, "axon_start_nrt_profile"):
            lib.axon_start_nrt_profile.argtypes = [
                ctypes.POINTER(ctypes.c_int64),
                ctypes.c_size_t,
            ]
            lib.axon_start_nrt_profile.restype = ctypes.c_int64
            lib.axon_stop_nrt_profile.argtypes = [ctypes.c_char_p]
            lib.axon_stop_nrt_profile.restype = ctypes.c_int64

            @contextlib.contextmanager
            def _hook(output_dir, device_ids):
                import jax

                jax.devices()
                if device_ids:
                    ids = (ctypes.c_int64 * len(device_ids))(*device_ids)
                    rc = lib.axon_start_nrt_profile(ids, len(device_ids))
                else:
                    rc = lib.axon_start_nrt_profile(None, 0)
                if rc != 0:
                    raise RuntimeError(f"axon_start_nrt_profile rc={rc}")
                try:
                    yield
                finally:
                    n = lib.axon_stop_nrt_profile(str(output_dir).encode())
                    print(f"profile: {n} file(s) written to {output_dir}")

            holder["hook"] = _hook

    sys.modules["antenv.axon_hooks"] = mod


def _prepare_inputs(x, Ws, bs, pW, pb, gW, gb):
    bf16 = ml_dtypes.bfloat16
    x = np.ascontiguousarray(np.asarray(x, np.float32))
    Ws = np.asarray(Ws, np.float32)
    bs = np.ascontiguousarray(np.asarray(bs, np.float32))
    pW = np.ascontiguousarray(np.asarray(pW, np.float32))
    pb = np.asarray(pb, np.float32)
    gW = np.ascontiguousarray(np.asarray(gW, np.float32))
    gb = np.ascontiguousarray(np.asarray(gb, np.float32))

    # W[(m, j), i] laid out as [ic, m, jj, jt, ii] so each (ic, m) block is a
    # contiguous 2 MB DMA with 16 KB per partition.
    w_host = np.ascontiguousarray(
        Ws.reshape(M, IC, 512, JT, 128).transpose(1, 0, 4, 3, 2)
    ).astype(bf16)

    in_maps = []
    for c in range(NCORES):
        rb = slice(c * BS, (c + 1) * BS)
        xc = x[rb]                                   # [512, 2048]
        xt = np.ascontiguousarray(
            xc.T.reshape(JT, 128, BS).transpose(1, 0, 2)
        ).astype(bf16)                               # [jj, jt, b]
        pbs_host = np.zeros((128, DO + BS), np.float32)
        pbs_host[:M, :DO] = bs
        pbs_host[:M, DO:] = pb[rb].T
        in_maps.append(
            {
                "w": w_host,
                "xt": xt,
                "pw": np.ascontiguousarray(pW[rb]),
                "pbs": pbs_host,
                "gw": np.ascontiguousarray(gW[rb]),
                "gb": np.ascontiguousarray(gb[rb]),
            }
        )
    return in_maps


def kernel(x, Ws, bs, pW, pb, gW, gb):
    global _PROGRAM, LAST_EXEC_NS, LAST_RESULTS
    from concourse.bass_utils import run_bass_kernel_spmd

    if os.environ.get("KERNEL_TRACE", "0") == "1":
        _install_ntff_shim()

    if _PROGRAM is None:
        _PROGRAM = _build_program()

    in_maps = _prepare_inputs(x, Ws, bs, pW, pb, gW, gb)
    trace = os.environ.get("KERNEL_TRACE", "0") == "1"
    res = run_bass_kernel_spmd(
        _PROGRAM, in_maps, core_ids=list(range(NCORES)), trace=trace
    )
    LAST_RESULTS = res
    LAST_EXEC_NS = res.exec_time_ns
    return np.concatenate([res.results[c]["out"] for c in range(NCORES)], axis=0)


# revision 13
# speedup vs baseline: 1.1012x; 1.0522x over previous
import os

import numpy as np
import ml_dtypes

# nn_GateModLinear: B=4096, M=8 experts, DI=DO=2048, LN eps=1e-5.
#   h[b,m,i] = sum_j Ws[m,i,j] x[b,j]
#   Wx = gW * sum_m pW[b,m] h[b,m,:]
#   z  = Wx + gb * (pb @ bs)
#   out = ELU(LayerNorm(z))
#
# Strategy: data-parallel over batch across 8 NeuronCores (512 rows each).
# Per core: one bf16 [512,16384]x[16384,2048] matmul on TensorE, streamed
# over W once; per-expert PSUM tiles are mixed with pW on ScalarE (per-
# partition scale) + VectorE adds; bias path is a K=128 zero-padded matmul;
# gating, LayerNorm and ELU run on VectorE/ScalarE. ELU uses
#   elu(y) = relu(y) + min(exp(y)-1, 0).
# The last output-column pass is split into two batch halves so half of the
# LN/ELU epilogue overlaps the tail of the matmul stream.

B, M, DI, DO = 4096, 8, 2048, 2048
NCORES = 8
BS = B // NCORES      # 512 batch rows per core
BT = BS // 128        # 4 b-tiles of 128 (partition dim)
IC = DO // 512        # 4 output chunks of 512
JT = DI // 128        # 16 contraction tiles of 128 per expert
LN_EPS = 1e-5

_PROGRAM = None          # cached bass.Bass program
LAST_EXEC_NS = None
LAST_RESULTS = None


def _build_program():
    import concourse.bacc as bacc
    import concourse.bass as bass
    import concourse.tile as tile
    import concourse.mybir as mybir

    f32 = mybir.dt.float32
    bf16 = mybir.dt.bfloat16
    AF = mybir.ActivationFunctionType
    OP = mybir.AluOpType

    nc = bacc.Bacc("TRN2")

    # [ic, m, jj, jt, ii]: W[(m, jt*128+jj), ic*512+ii] = Ws[m, i, j]
    w_d = nc.dram_tensor("w", [IC, M, 128, JT, 512], bf16, kind="ExternalInput")
    # [jj, jt, b]: xT for this core, k=jt*128+jj on partitions
    xt_d = nc.dram_tensor("xt", [128, JT, BS], bf16, kind="ExternalInput")
    pw_d = nc.dram_tensor("pw", [BS, M], f32, kind="ExternalInput")
    # [:, :DO] = bs zero-padded to 128 rows; [:, DO:] = pb.T zero-padded.
    pbs_d = nc.dram_tensor("pbs", [128, DO + BS], bf16, kind="ExternalInput")
    gw_d = nc.dram_tensor("gw", [BS, DO], f32, kind="ExternalInput")
    gb_d = nc.dram_tensor("gb", [BS, DO], bf16, kind="ExternalInput")
    out_d = nc.dram_tensor("out", [BS, DO], f32, kind="ExternalOutput")

    with tile.TileContext(nc) as tc:
        with (
            tc.tile_pool(name="const", bufs=1) as cpool,
            tc.tile_pool(name="wpool", bufs=2) as wpool,
            tc.tile_pool(name="acc", bufs=1) as apool,
            tc.tile_pool(name="mix", bufs=3) as mpool,
            tc.tile_pool(name="gio", bufs=4) as gpool,
            tc.tile_pool(name="epi", bufs=1) as epool,
            tc.tile_pool(name="psum", bufs=2, space=bass.MemorySpace.PSUM) as pspool,
        ):
            # ---- constants / persistent tiles ----
            # pbs first: the bias matmuls depend only on it, so PE starts
            # (and HAM-warms) as early as possible.
            pbs_s = cpool.tile([128, DO + BS], bf16)
            nc.sync.dma_start(out=pbs_s[:, DO:], in_=pbs_d[:, DO:])
            for c0 in range(0, DO, 512):
                nc.sync.dma_start(out=pbs_s[:, c0:c0 + 512],
                                  in_=pbs_d[:, c0:c0 + 512])
            xt_s = cpool.tile([128, JT, BS], bf16)
            nc.scalar.dma_start(out=xt_s[:], in_=xt_d[:])
            pw_t = []
            for bt in range(BT):
                t = cpool.tile([128, M], f32, name=f"pw{bt}", tag=f"pw{bt}")
                nc.sync.dma_start(out=t[:], in_=pw_d[bt * 128:(bt + 1) * 128, :])
                pw_t.append(t)
            eps_t = cpool.tile([128, 1], f32, name="eps", tag="eps")
            nc.vector.memset(eps_t[:], LN_EPS)

            z_t = [apool.tile([128, DO], f32, name=f"z{bt}", tag=f"z{bt}")
                   for bt in range(BT)]
            zraw = [apool.tile([128, DO], bf16, name=f"zr{bt}", tag=f"zr{bt}")
                    for bt in range(BT)]
            wx_t = [apool.tile([128, DO], f32, name=f"wx{bt}", tag=f"wx{bt}")
                    for bt in range(BT)]
            st_t = [apool.tile([128, IC, 6], f32, name=f"st{bt}", tag=f"st{bt}")
                    for bt in range(BT)]

            # ---- phase A: zraw = pb @ bs (bias path, K padded to 128) ----
            # Depends only on pbs, so it starts immediately and warms the PE;
            # the gb gate is applied later in combine.
            for bt in range(BT):
                for ic in range(IC):
                    ps = pspool.tile([128, 512], f32, name=f"ps{bt}", tag=f"b{bt}")
                    nc.tensor.matmul(
                        ps[:],
                        pbs_s[:, DO + bt * 128:DO + (bt + 1) * 128],
                        pbs_s[:, ic * 512:(ic + 1) * 512],
                        start=True,
                        stop=True,
                    )
                    nc.scalar.activation(
                        zraw[bt][:, ic * 512:(ic + 1) * 512], ps[:], AF.Copy
                    )

            def mix(ic, m, bt, ps):
                wxs = wx_t[bt][:, ic * 512:(ic + 1) * 512]
                if m == 0:
                    nc.scalar.activation(
                        wxs, ps[:], AF.Copy, scale=pw_t[bt][:, 0:1]
                    )
                else:
                    tmp = mpool.tile([128, 512], f32, name="tmp", tag="tmp")
                    nc.scalar.activation(
                        tmp[:], ps[:], AF.Copy, scale=pw_t[bt][:, m:m + 1]
                    )
                    nc.vector.tensor_add(wxs, wxs, tmp[:])

            def load_w(ic, m):
                w_s = wpool.tile([128, JT, 512], bf16, name="ws", tag="w")
                eng = nc.sync if (ic * M + m) % 2 == 0 else nc.scalar
                for j0 in range(0, JT, 8):
                    eng.dma_start(out=w_s[:, j0:j0 + 8, :],
                                  in_=w_d[ic, m, :, j0:j0 + 8, :])
                return w_s

            def mm_group(ic, m, bts, bt_outer=False, w_s=None):
                """One (ic, m) accumulation group over the given b-tiles."""
                if w_s is None:
                    w_s = load_w(ic, m)
                pss = {
                    bt: pspool.tile([128, 512], f32, name=f"ps{bt}", tag=f"b{bt}")
                    for bt in bts
                }
                if bt_outer:
                    # serialize per b-tile so early tiles close (and mix)
                    # while later tiles are still on the PE
                    for bt in bts:
                        for jt in range(JT):
                            nc.tensor.matmul(
                                pss[bt][:],
                                xt_s[:, jt, bt * 128:(bt + 1) * 128],
                                w_s[:, jt, :],
                                start=(jt == 0),
                                stop=(jt == JT - 1),
                            )
                        mix(ic, m, bt, pss[bt])
                    return
                for jt in range(JT):
                    for bt in bts:
                        nc.tensor.matmul(
                            pss[bt][:],
                            xt_s[:, jt, bt * 128:(bt + 1) * 128],
                            w_s[:, jt, :],
                            start=(jt == 0),
                            stop=(jt == JT - 1),
                        )
                for bt in bts:
                    mix(ic, m, bt, pss[bt])

            def combine(ic, bts):
                """z[:, ic] = gW * wx + gb * zraw and bn_stats for the chunk."""
                for bt in bts:
                    gwt = gpool.tile([128, 512], f32, name="gwt", tag="gw")
                    nc.sync.dma_start(
                        out=gwt[:],
                        in_=gw_d[bt * 128:(bt + 1) * 128, ic * 512:(ic + 1) * 512],
                    )
                    gbt = gpool.tile([128, 512], bf16, name="gbt", tag="gb")
                    nc.scalar.dma_start(
                        out=gbt[:],
                        in_=gb_d[bt * 128:(bt + 1) * 128, ic * 512:(ic + 1) * 512],
                    )
                    wxs = wx_t[bt][:, ic * 512:(ic + 1) * 512]
                    zs = z_t[bt][:, ic * 512:(ic + 1) * 512]
                    tmp2 = mpool.tile([128, 512], f32, name="tmp2", tag="tmp2")
                    nc.vector.tensor_mul(tmp2[:], gwt[:], wxs)
                    t3 = mpool.tile([128, 512], f32, name="t3", tag="t3")
                    nc.vector.tensor_mul(
                        t3[:], gbt[:], zraw[bt][:, ic * 512:(ic + 1) * 512]
                    )
                    nc.vector.tensor_add(zs, tmp2[:], t3[:])
                    nc.vector.bn_stats(st_t[bt][:, ic, :], zs)

            def epilogue(bts):
                """LayerNorm + ELU + store for the given b-tiles."""
                mvs, rstds = {}, {}
                for bt in bts:
                    mv = mpool.tile([128, 2], f32, name="mv", tag=f"mv{bt}")
                    nc.vector.bn_aggr(mv[:], st_t[bt][:])
                    mvs[bt] = mv
                for bt in bts:  # grouped: one ACT Sqrt table load
                    sd = mpool.tile([128, 1], f32, name="sd", tag=f"sd{bt}")
                    nc.scalar.activation(sd[:], mvs[bt][:, 1:2], AF.Sqrt,
                                         bias=eps_t[:])
                    rstds[bt] = sd
                for bt in bts:
                    nc.vector.reciprocal(rstds[bt][:], rstds[bt][:])
                ys, es = {}, {}
                for bt in bts:
                    y = epool.tile([128, DO], f32, name="y", tag=f"y{bt % 2}")
                    nc.vector.tensor_scalar(
                        out=y[:],
                        in0=z_t[bt][:],
                        scalar1=mvs[bt][:, 0:1],
                        scalar2=rstds[bt][:],
                        op0=OP.subtract,
                        op1=OP.mult,
                    )
                    ys[bt] = y
                for bt in bts:  # grouped: one ACT Exp table load
                    e = epool.tile([128, DO], bf16, name="e", tag=f"e{bt % 2}")
                    nc.scalar.activation(e[:], ys[bt][:], AF.Exp)
                    es[bt] = e
                for bt in bts:
                    y, e = ys[bt], es[bt]
                    nc.vector.tensor_scalar_max(y[:], y[:], 0.0)
                    nc.vector.tensor_scalar(
                        out=e[:],
                        in0=e[:],
                        scalar1=1.0,
                        scalar2=1.0,
                        op0=OP.min,
                        op1=OP.subtract,
                    )
                    nc.vector.tensor_add(y[:], y[:], e[:])
                    nc.scalar.dma_start(
                        out=out_d[bt * 128:(bt + 1) * 128, :], in_=y[:]
                    )

            # ---- phase B: main matmul + expert mix + gate + stats ----
            for ic in range(IC):
                last = ic == IC - 1
                for m in range(M - 1 if last else M):
                    mm_group(ic, m, range(BT))
                if not last:
                    combine(ic, range(BT))
            # Final expert group runs b-tile-serial so each pair's gate/LN/ELU
            # overlaps the remaining matmuls.
            w_last = load_w(IC - 1, M - 1)
            for half in ((0, 1), (2, 3)):
                mm_group(IC - 1, M - 1, half, bt_outer=True, w_s=w_last)
                combine(IC - 1, half)
                epilogue(half)

    nc.compile()
    return nc


def _install_ntff_shim():
    """Provide antenv.axon_hooks (NTFF profiling hook) if the image lacks it.

    Mirrors the ctypes hook normally installed at boot: drives
    axon_{start,stop}_nrt_profile in libaxon_pjrt.so so run_bass_kernel_spmd
    trace=True can capture per-core NTFF profiles."""
    import sys
    import types
    import ctypes
    import contextlib

    try:
        from antenv.axon_hooks import get_axon_ntff_profile_hook  # noqa: F401
        return
    except ImportError:
        pass

    holder = {"hook": None}
    mod = types.ModuleType("antenv.axon_hooks")
    mod.set_axon_ntff_profile_hook = lambda h: holder.__setitem__("hook", h)
    mod.get_axon_ntff_profile_hook = lambda: holder["hook"]

    so_path = "/opt/axon/libaxon_pjrt.so"
    if os.path.exists(so_path):
        lib = ctypes.CDLL(so_path)
        if hasattr(lib, "axon_start_nrt_profile"):
            lib.axon_start_nrt_profile.argtypes = [
                ctypes.POINTER(ctypes.c_int64),
                ctypes.c_size_t,
            ]
            lib.axon_start_nrt_profile.restype = ctypes.c_int64
            lib.axon_stop_nrt_profile.argtypes = [ctypes.c_char_p]
            lib.axon_stop_nrt_profile.restype = ctypes.c_int64

            @contextlib.contextmanager
            def _hook(output_dir, device_ids):
                import jax

                jax.devices()
                if device_ids:
                    ids = (ctypes.c_int64 * len(device_ids))(*device_ids)
                    rc = lib.axon_start_nrt_profile(ids, len(device_ids))
                else:
                    rc = lib.axon_start_nrt_profile(None, 0)
                if rc != 0:
                    raise RuntimeError(f"axon_start_nrt_profile rc={rc}")
                try:
                    yield
                finally:
                    n = lib.axon_stop_nrt_profile(str(output_dir).encode())
                    print(f"profile: {n} file(s) written to {output_dir}")

            holder["hook"] = _hook

    sys.modules["antenv.axon_hooks"] = mod


def _prepare_inputs(x, Ws, bs, pW, pb, gW, gb):
    bf16 = ml_dtypes.bfloat16
    x = np.ascontiguousarray(np.asarray(x, np.float32))
    Ws = np.asarray(Ws, np.float32)
    bs = np.ascontiguousarray(np.asarray(bs, np.float32))
    pW = np.ascontiguousarray(np.asarray(pW, np.float32))
    pb = np.asarray(pb, np.float32)
    gW = np.ascontiguousarray(np.asarray(gW, np.float32))
    gb = np.ascontiguousarray(np.asarray(gb, np.float32))

    # W[(m, j), i] laid out as [ic, m, jj, jt, ii] so each (ic, m) block is a
    # contiguous 2 MB DMA with 16 KB per partition.
    w_host = np.ascontiguousarray(
        Ws.reshape(M, IC, 512, JT, 128).transpose(1, 0, 4, 3, 2)
    ).astype(bf16)

    in_maps = []
    for c in range(NCORES):
        rb = slice(c * BS, (c + 1) * BS)
        xc = x[rb]                                   # [512, 2048]
        xt = np.ascontiguousarray(
            xc.T.reshape(JT, 128, BS).transpose(1, 0, 2)
        ).astype(bf16)                               # [jj, jt, b]
        pbs_host = np.zeros((128, DO + BS), bf16)
        pbs_host[:M, :DO] = bs.astype(bf16)
        pbs_host[:M, DO:] = pb[rb].T.astype(bf16)
        in_maps.append(
            {
                "w": w_host,
                "xt": xt,
                "pw": np.ascontiguousarray(pW[rb]),
                "pbs": pbs_host,
                "gw": np.ascontiguousarray(gW[rb]),
                "gb": np.ascontiguousarray(gb[rb]).astype(bf16),
            }
        )
    return in_maps


def kernel(x, Ws, bs, pW, pb, gW, gb):
    global _PROGRAM, LAST_EXEC_NS, LAST_RESULTS
    from concourse.bass_utils import run_bass_kernel_spmd

    if os.environ.get("KERNEL_TRACE", "0") == "1":
        _install_ntff_shim()

    if _PROGRAM is None:
        _PROGRAM = _build_program()

    in_maps = _prepare_inputs(x, Ws, bs, pW, pb, gW, gb)
    trace = os.environ.get("KERNEL_TRACE", "0") == "1"
    res = run_bass_kernel_spmd(
        _PROGRAM, in_maps, core_ids=list(range(NCORES)), trace=trace
    )
    LAST_RESULTS = res
    LAST_EXEC_NS = res.exec_time_ns
    return np.concatenate([res.results[c]["out"] for c in range(NCORES)], axis=0)


# revision 14
# speedup vs baseline: 1.1132x; 1.0109x over previous
import os

import numpy as np
import ml_dtypes

# nn_GateModLinear: B=4096, M=8 experts, DI=DO=2048, LN eps=1e-5.
#   h[b,m,i] = sum_j Ws[m,i,j] x[b,j]
#   Wx = gW * sum_m pW[b,m] h[b,m,:]
#   z  = Wx + gb * (pb @ bs)
#   out = ELU(LayerNorm(z))
#
# Strategy: data-parallel over batch across 8 NeuronCores (512 rows each).
# Per core: one bf16 [512,16384]x[16384,2048] matmul on TensorE, streamed
# over W once; per-expert PSUM tiles are mixed with pW on ScalarE (per-
# partition scale) + VectorE adds; bias path is a K=128 zero-padded matmul;
# gating, LayerNorm and ELU run on VectorE/ScalarE. ELU uses
#   elu(y) = relu(y) + min(exp(y)-1, 0).
# The last output-column pass is split into two batch halves so half of the
# LN/ELU epilogue overlaps the tail of the matmul stream.

B, M, DI, DO = 4096, 8, 2048, 2048
NCORES = 8
BS = B // NCORES      # 512 batch rows per core
BT = BS // 128        # 4 b-tiles of 128 (partition dim)
IC = DO // 512        # 4 output chunks of 512
JT = DI // 128        # 16 contraction tiles of 128 per expert
LN_EPS = 1e-5

_PROGRAM = None          # cached bass.Bass program
LAST_EXEC_NS = None
LAST_RESULTS = None


def _build_program():
    import concourse.bacc as bacc
    import concourse.bass as bass
    import concourse.tile as tile
    import concourse.mybir as mybir

    f32 = mybir.dt.float32
    bf16 = mybir.dt.bfloat16
    AF = mybir.ActivationFunctionType
    OP = mybir.AluOpType

    nc = bacc.Bacc("TRN2")

    # [ic, m, jj, jt, ii]: W[(m, jt*128+jj), ic*512+ii] = Ws[m, i, j]
    w_d = nc.dram_tensor("w", [IC, M, 128, JT, 512], bf16, kind="ExternalInput")
    # [jj, jt, b]: xT for this core, k=jt*128+jj on partitions
    xt_d = nc.dram_tensor("xt", [128, JT, BS], bf16, kind="ExternalInput")
    pw_d = nc.dram_tensor("pw", [BS, M], f32, kind="ExternalInput")
    # [:, :DO] = bs zero-padded to 128 rows; [:, DO:] = pb.T zero-padded.
    pbs_d = nc.dram_tensor("pbs", [128, DO + BS], bf16, kind="ExternalInput")
    gw_d = nc.dram_tensor("gw", [BS, DO], f32, kind="ExternalInput")
    gb_d = nc.dram_tensor("gb", [BS, DO], bf16, kind="ExternalInput")
    out_d = nc.dram_tensor("out", [BS, DO], bf16, kind="ExternalOutput")

    with tile.TileContext(nc) as tc:
        with (
            tc.tile_pool(name="const", bufs=1) as cpool,
            tc.tile_pool(name="wpool", bufs=2) as wpool,
            tc.tile_pool(name="acc", bufs=1) as apool,
            tc.tile_pool(name="mix", bufs=3) as mpool,
            tc.tile_pool(name="gio", bufs=4) as gpool,
            tc.tile_pool(name="epi", bufs=1) as epool,
            tc.tile_pool(name="psum", bufs=2, space=bass.MemorySpace.PSUM) as pspool,
        ):
            # ---- constants / persistent tiles ----
            # pbs first: the bias matmuls depend only on it, so PE starts
            # (and HAM-warms) as early as possible.
            pbs_s = cpool.tile([128, DO + BS], bf16)
            nc.sync.dma_start(out=pbs_s[:, DO:], in_=pbs_d[:, DO:])
            for i, c0 in enumerate(range(0, DO, 512)):
                eng = nc.sync if i % 2 == 0 else nc.scalar
                eng.dma_start(out=pbs_s[:, c0:c0 + 512],
                              in_=pbs_d[:, c0:c0 + 512])
            xt_s = cpool.tile([128, JT, BS], bf16)
            nc.scalar.dma_start(out=xt_s[:], in_=xt_d[:])
            pw_t = []
            for bt in range(BT):
                t = cpool.tile([128, M], f32, name=f"pw{bt}", tag=f"pw{bt}")
                nc.sync.dma_start(out=t[:], in_=pw_d[bt * 128:(bt + 1) * 128, :])
                pw_t.append(t)
            eps_t = cpool.tile([128, 1], f32, name="eps", tag="eps")
            nc.vector.memset(eps_t[:], LN_EPS)

            # Warm the PE clock (HAM) with throwaway matmuls while the first
            # input DMAs are still in flight.
            wdum = cpool.tile([128, 512], bf16, name="wdum", tag="wdum")
            nc.vector.memset(wdum[:], 0.0)
            for i in range(12):
                pd = pspool.tile([128, 512], f32, name="pdum", tag=f"b{i % 4}")
                nc.tensor.matmul(pd[:], wdum[:, 0:128], wdum[:],
                                 start=True, stop=True)

            z_t = [apool.tile([128, DO], f32, name=f"z{bt}", tag=f"z{bt}")
                   for bt in range(BT)]
            zraw = [apool.tile([128, DO], bf16, name=f"zr{bt}", tag=f"zr{bt}")
                    for bt in range(BT)]
            wx_t = [apool.tile([128, DO], f32, name=f"wx{bt}", tag=f"wx{bt}")
                    for bt in range(BT)]
            st_t = [apool.tile([128, IC, 6], f32, name=f"st{bt}", tag=f"st{bt}")
                    for bt in range(BT)]

            # ---- phase A: zraw = pb @ bs (bias path, K padded to 128) ----
            # Depends only on pbs, so it starts immediately and warms the PE;
            # the gb gate is applied later in combine.
            for bt in range(BT):
                for ic in range(IC):
                    ps = pspool.tile([128, 512], f32, name=f"ps{bt}", tag=f"b{bt}")
                    nc.tensor.matmul(
                        ps[:],
                        pbs_s[:, DO + bt * 128:DO + (bt + 1) * 128],
                        pbs_s[:, ic * 512:(ic + 1) * 512],
                        start=True,
                        stop=True,
                    )
                    nc.scalar.activation(
                        zraw[bt][:, ic * 512:(ic + 1) * 512], ps[:], AF.Copy
                    )

            def mix(ic, m, bt, ps):
                wxs = wx_t[bt][:, ic * 512:(ic + 1) * 512]
                if m == 0:
                    nc.scalar.activation(
                        wxs, ps[:], AF.Copy, scale=pw_t[bt][:, 0:1]
                    )
                else:
                    tmp = mpool.tile([128, 512], f32, name="tmp", tag="tmp")
                    nc.scalar.activation(
                        tmp[:], ps[:], AF.Copy, scale=pw_t[bt][:, m:m + 1]
                    )
                    nc.vector.tensor_add(wxs, wxs, tmp[:])

            def load_w(ic, m):
                w_s = wpool.tile([128, JT, 512], bf16, name="ws", tag="w")
                eng = nc.sync if (ic * M + m) % 2 == 0 else nc.scalar
                for j0 in range(0, JT, 8):
                    eng.dma_start(out=w_s[:, j0:j0 + 8, :],
                                  in_=w_d[ic, m, :, j0:j0 + 8, :])
                return w_s

            def mm_group(ic, m, bts, bt_outer=False, w_s=None):
                """One (ic, m) accumulation group over the given b-tiles."""
                if w_s is None:
                    w_s = load_w(ic, m)
                pss = {
                    bt: pspool.tile([128, 512], f32, name=f"ps{bt}", tag=f"b{bt}")
                    for bt in bts
                }
                if bt_outer:
                    # serialize per b-tile so early tiles close (and mix)
                    # while later tiles are still on the PE
                    for bt in bts:
                        for jt in range(JT):
                            nc.tensor.matmul(
                                pss[bt][:],
                                xt_s[:, jt, bt * 128:(bt + 1) * 128],
                                w_s[:, jt, :],
                                start=(jt == 0),
                                stop=(jt == JT - 1),
                            )
                        mix(ic, m, bt, pss[bt])
                    return
                for jt in range(JT):
                    for bt in bts:
                        nc.tensor.matmul(
                            pss[bt][:],
                            xt_s[:, jt, bt * 128:(bt + 1) * 128],
                            w_s[:, jt, :],
                            start=(jt == 0),
                            stop=(jt == JT - 1),
                        )
                for bt in bts:
                    mix(ic, m, bt, pss[bt])

            def combine(ic, bts):
                """z[:, ic] = gW * wx + gb * zraw and bn_stats for the chunk."""
                for bt in bts:
                    gwt = gpool.tile([128, 512], f32, name="gwt", tag="gw")
                    nc.sync.dma_start(
                        out=gwt[:],
                        in_=gw_d[bt * 128:(bt + 1) * 128, ic * 512:(ic + 1) * 512],
                    )
                    gbt = gpool.tile([128, 512], bf16, name="gbt", tag="gb")
                    nc.scalar.dma_start(
                        out=gbt[:],
                        in_=gb_d[bt * 128:(bt + 1) * 128, ic * 512:(ic + 1) * 512],
                    )
                    wxs = wx_t[bt][:, ic * 512:(ic + 1) * 512]
                    zs = z_t[bt][:, ic * 512:(ic + 1) * 512]
                    tmp2 = mpool.tile([128, 512], f32, name="tmp2", tag="tmp2")
                    nc.vector.tensor_mul(tmp2[:], gwt[:], wxs)
                    t3 = mpool.tile([128, 512], f32, name="t3", tag="t3")
                    nc.vector.tensor_mul(
                        t3[:], gbt[:], zraw[bt][:, ic * 512:(ic + 1) * 512]
                    )
                    nc.vector.tensor_add(zs, tmp2[:], t3[:])
                    nc.vector.bn_stats(st_t[bt][:, ic, :], zs)

            def epilogue(bts):
                """LayerNorm + ELU + store for the given b-tiles."""
                mvs, rstds = {}, {}
                for bt in bts:
                    mv = mpool.tile([128, 2], f32, name="mv", tag=f"mv{bt}")
                    nc.vector.bn_aggr(mv[:], st_t[bt][:])
                    mvs[bt] = mv
                for bt in bts:  # grouped: one ACT Sqrt table load
                    sd = mpool.tile([128, 1], f32, name="sd", tag=f"sd{bt}")
                    nc.scalar.activation(sd[:], mvs[bt][:, 1:2], AF.Sqrt,
                                         bias=eps_t[:])
                    rstds[bt] = sd
                for bt in bts:
                    nc.vector.reciprocal(rstds[bt][:], rstds[bt][:])
                ys, es = {}, {}
                for bt in bts:
                    y = epool.tile([128, DO], bf16, name="y", tag=f"y{bt % 2}")
                    nc.vector.tensor_scalar(
                        out=y[:],
                        in0=z_t[bt][:],
                        scalar1=mvs[bt][:, 0:1],
                        scalar2=rstds[bt][:],
                        op0=OP.subtract,
                        op1=OP.mult,
                    )
                    ys[bt] = y
                for bt in bts:  # grouped: one ACT Exp table load
                    e = epool.tile([128, DO], bf16, name="e", tag=f"e{bt % 2}")
                    nc.scalar.activation(e[:], ys[bt][:], AF.Exp)
                    es[bt] = e
                for bt in bts:
                    y, e = ys[bt], es[bt]
                    nc.vector.tensor_scalar_max(y[:], y[:], 0.0)
                    nc.vector.tensor_scalar(
                        out=e[:],
                        in0=e[:],
                        scalar1=1.0,
                        scalar2=1.0,
                        op0=OP.min,
                        op1=OP.subtract,
                    )
                    nc.vector.tensor_add(y[:], y[:], e[:])
                    nc.scalar.dma_start(
                        out=out_d[bt * 128:(bt + 1) * 128, :], in_=y[:]
                    )

            # ---- phase B: main matmul + expert mix + gate + stats ----
            for ic in range(IC):
                last = ic == IC - 1
                for m in range(M - 1 if last else M):
                    mm_group(ic, m, range(BT))
                if not last:
                    combine(ic, range(BT))
            # Final expert group runs b-tile-serial so each pair's gate/LN/ELU
            # overlaps the remaining matmuls.
            w_last = load_w(IC - 1, M - 1)
            for half in ((0, 1), (2, 3)):
                mm_group(IC - 1, M - 1, half, bt_outer=True, w_s=w_last)
                combine(IC - 1, half)
                epilogue(half)

    nc.compile()
    return nc


def _install_ntff_shim():
    """Provide antenv.axon_hooks (NTFF profiling hook) if the image lacks it.

    Mirrors the ctypes hook normally installed at boot: drives
    axon_{start,stop}_nrt_profile in libaxon_pjrt.so so run_bass_kernel_spmd
    trace=True can capture per-core NTFF profiles."""
    import sys
    import types
    import ctypes
    import contextlib

    try:
        from antenv.axon_hooks import get_axon_ntff_profile_hook  # noqa: F401
        return
    except ImportError:
        pass

    holder = {"hook": None}
    mod = types.ModuleType("antenv.axon_hooks")
    mod.set_axon_ntff_profile_hook = lambda h: holder.__setitem__("hook", h)
    mod.get_axon_ntff_profile_hook = lambda: holder["hook"]

    so_path = "/opt/axon/libaxon_pjrt.so"
    if os.path.exists(so_path):
        lib = ctypes.CDLL(so_path)
        if hasattr(lib, "axon_start_nrt_profile"):
            lib.axon_start_nrt_profile.argtypes = [
                ctypes.POINTER(ctypes.c_int64),
                ctypes.c_size_t,
            ]
            lib.axon_start_nrt_profile.restype = ctypes.c_int64
            lib.axon_stop_nrt_profile.argtypes = [ctypes.c_char_p]
            lib.axon_stop_nrt_profile.restype = ctypes.c_int64

            @contextlib.contextmanager
            def _hook(output_dir, device_ids):
                import jax

                jax.devices()
                if device_ids:
                    ids = (ctypes.c_int64 * len(device_ids))(*device_ids)
                    rc = lib.axon_start_nrt_profile(ids, len(device_ids))
                else:
                    rc = lib.axon_start_nrt_profile(None, 0)
                if rc != 0:
                    raise RuntimeError(f"axon_start_nrt_profile rc={rc}")
                try:
                    yield
                finally:
                    n = lib.axon_stop_nrt_profile(str(output_dir).encode())
                    print(f"profile: {n} file(s) written to {output_dir}")

            holder["hook"] = _hook

    sys.modules["antenv.axon_hooks"] = mod


def _prepare_inputs(x, Ws, bs, pW, pb, gW, gb):
    bf16 = ml_dtypes.bfloat16
    x = np.ascontiguousarray(np.asarray(x, np.float32))
    Ws = np.asarray(Ws, np.float32)
    bs = np.ascontiguousarray(np.asarray(bs, np.float32))
    pW = np.ascontiguousarray(np.asarray(pW, np.float32))
    pb = np.asarray(pb, np.float32)
    gW = np.ascontiguousarray(np.asarray(gW, np.float32))
    gb = np.ascontiguousarray(np.asarray(gb, np.float32))

    # W[(m, j), i] laid out as [ic, m, jj, jt, ii] so each (ic, m) block is a
    # contiguous 2 MB DMA with 16 KB per partition.
    w_host = np.ascontiguousarray(
        Ws.reshape(M, IC, 512, JT, 128).transpose(1, 0, 4, 3, 2)
    ).astype(bf16)

    in_maps = []
    for c in range(NCORES):
        rb = slice(c * BS, (c + 1) * BS)
        xc = x[rb]                                   # [512, 2048]
        xt = np.ascontiguousarray(
            xc.T.reshape(JT, 128, BS).transpose(1, 0, 2)
        ).astype(bf16)                               # [jj, jt, b]
        pbs_host = np.zeros((128, DO + BS), bf16)
        pbs_host[:M, :DO] = bs.astype(bf16)
        pbs_host[:M, DO:] = pb[rb].T.astype(bf16)
        in_maps.append(
            {
                "w": w_host,
                "xt": xt,
                "pw": np.ascontiguousarray(pW[rb]),
                "pbs": pbs_host,
                "gw": np.ascontiguousarray(gW[rb]),
                "gb": np.ascontiguousarray(gb[rb]).astype(bf16),
            }
        )
    return in_maps


def kernel(x, Ws, bs, pW, pb, gW, gb):
    global _PROGRAM, LAST_EXEC_NS, LAST_RESULTS
    from concourse.bass_utils import run_bass_kernel_spmd

    if os.environ.get("KERNEL_TRACE", "0") == "1":
        _install_ntff_shim()

    if _PROGRAM is None:
        _PROGRAM = _build_program()

    in_maps = _prepare_inputs(x, Ws, bs, pW, pb, gW, gb)
    trace = os.environ.get("KERNEL_TRACE", "0") == "1"
    res = run_bass_kernel_spmd(
        _PROGRAM, in_maps, core_ids=list(range(NCORES)), trace=trace
    )
    LAST_RESULTS = res
    LAST_EXEC_NS = res.exec_time_ns
    return np.concatenate(
        [res.results[c]["out"] for c in range(NCORES)], axis=0
    ).astype(np.float32)
